# revision 19
# baseline (speedup 1.0000x reference)
"""Binary associative memory (causal linear attention with binarized k/v).

Self-contained Trainium2 Bass kernel.

Math: the reference's chunked prefix recurrence telescopes to exact causal
linear attention:
    out[t] = (1/(8*(t+1))) * sum_{s<=t} (q[t].k[s]) v[s],   k,v = sign(qkv)
    y      = out @ W_o.T   (summed over head features)
    final_matrix[b,h] = sum_t k[t] (x) v[t]   (exact integers)
so we are free to re-chunk at 128 tokens (partition width).

Sharding: 8 cores = 4 batches x 2 head-groups (8 heads each).

Precision: k/v binarize must match the fp32 reference's signs, so the k/v
projections use an fp16 double-double trick: x ~= x1 + x2 and W ~= w1 + w2
(fp16 splits, prepared on host) stacked along the contraction dim, so one
K=2048 accumulation computes (x1+x2)@(w1+w2) with ~1e-7 relative error at
full PE speed. q (continuous) uses a single fp16 term; o_proj runs fp16.

Layouts per core:
  - q,k projected feature-on-partition ([feat, tok]) for scoresT/crossT
  - v projected token-on-partition directly ([tok, vfeat]) for intra/ckv
  - k transposed per chunk on the PE (with identity) for ckv
  - state S[kf, df] fp16 (exact small integers), 2 heads per 128 partitions
"""

import functools

import numpy as np

T = 4096
D = 1024
HLOC = 8  # heads per core
DH = 64
CH = 128  # chunk size
NCH = T // CH  # 32
TT = 512  # projection token tile
NTT = T // TT  # 8

PAIRS = [(0, 2), (1, 3), (4, 6), (5, 7)]  # scores PSUM-tile pairs (same row-group)
# 3-term fp16 split of x@W: x1w1 + x2w1 + x1w2 (chunk indices into [hi(8), lo(8)])
SPLIT3 = [(wc, xc) for wc, xc in
          [(c, c) for c in range(8)] + [(c, c + 8) for c in range(8)] + [(c + 8, c) for c in range(8)]]


@functools.lru_cache(maxsize=1)
def _build():
    from contextlib import ExitStack

    import concourse.bacc as bacc
    import concourse.mybir as mybir
    import concourse.tile as tile

    f32 = mybir.dt.float32
    f16 = mybir.dt.float16

    nc = bacc.Bacc("TRN2", target_bir_lowering=False, debug=False, num_devices=8)

    # x split [x1; x2], chunk-major: [16 kc][128][T]
    xs = nc.dram_tensor("xs", [16, 128, T], f16, kind="ExternalInput").ap()
    wq = nc.dram_tensor("wq", [8, 128, 512], f16, kind="ExternalInput").ap()
    wk = nc.dram_tensor("wk", [16, 128, 512], f16, kind="ExternalInput").ap()
    wv = nc.dram_tensor("wv", [16, 128, 512], f16, kind="ExternalInput").ap()
    wo = nc.dram_tensor("wo", [4, 128, 1024], f16, kind="ExternalInput").ap()
    invtot = nc.dram_tensor("invtot", [128, T], f32, kind="ExternalInput").ap()
    mask2 = nc.dram_tensor("mask2", [128, 512], f32, kind="ExternalInput").ap()
    ident = nc.dram_tensor("ident", [128, 128], f16, kind="ExternalInput").ap()
    ypart = nc.dram_tensor("ypart", [T, D], f32, kind="ExternalOutput").ap()
    fmat = nc.dram_tensor("fmat", [HLOC, DH, DH], f32, kind="ExternalOutput").ap()

    with tile.TileContext(nc) as tc, ExitStack() as ctx:
        const = ctx.enter_context(tc.tile_pool(name="const", bufs=1))
        xpool = ctx.enter_context(tc.tile_pool(name="xp", bufs=3))
        qkpool = ctx.enter_context(tc.tile_pool(name="qk", bufs=2))
        tokpool = ctx.enter_context(tc.tile_pool(name="tok", bufs=3))
        stpool = ctx.enter_context(tc.tile_pool(name="st", bufs=8))
        opool = ctx.enter_context(tc.tile_pool(name="op", bufs=3))
        ypool = ctx.enter_context(tc.tile_pool(name="yp", bufs=3))
        spool = ctx.enter_context(tc.tile_pool(name="sp", bufs=2))
        pspool = ctx.enter_context(tc.tile_pool(name="ps", bufs=8, space="PSUM"))

        # weights needed by the first projections: emit DMAs first
        wq_sb = const.tile([128, 8, 512], f16)
        for kc in range(8):
            nc.sync.dma_start(wq_sb[:, kc, :], wq[kc])
        wk_sb = const.tile([128, 16, 512], f16)
        for kc in range(16):
            nc.sync.dma_start(wk_sb[:, kc, :], wk[kc])

        xs_tiles = {}

        def dma_x(tt, kc_lo, kc_hi):
            if tt >= NTT:
                return
            if tt not in xs_tiles:
                xs_tiles[tt] = xpool.tile([128, 16, TT], f16, tag="xs", name="xs_sb")
            t0 = tt * TT
            for kc in range(kc_lo, kc_hi):
                nc.sync.dma_start(xs_tiles[tt][:, kc, :], xs[kc, :, t0 : t0 + TT])

        qk_tiles = {}

        def proj_qk(tt, fb):
            """Build projection emitters for q/k feature-block fb of tile tt.

            Returns (stream, finish): stream is a list of matmul thunks to
            interleave with other PSUM banks; finish emits the evacuations."""
            if tt >= NTT:
                return [], lambda: None
            if tt not in qk_tiles:
                qk_tiles[tt] = (
                    qkpool.tile([128, 4 * TT], f16, tag="qT", name="qT_sb"),
                    qkpool.tile([128, 4 * TT], f16, tag="kT", name="kT_sb"),
                )
            qT, kT = qk_tiles[tt]
            xt = xs_tiles[tt]
            pq = pspool.tile([128, 512], f32, tag="work", name="pq")
            pk = pspool.tile([128, 512], f32, tag="work", name="pk")

            def mm_pq(kc, pq=pq, xt=xt, fb=fb):
                nc.tensor.matmul(
                    pq,
                    wq_sb[:, kc, fb * 128 : (fb + 1) * 128],
                    xt[:, kc, :],
                    start=(kc == 0),
                    stop=(kc == 7),
                )

            def mm_pk(i, pk=pk, xt=xt, fb=fb):
                wc, xc = SPLIT3[i]
                nc.tensor.matmul(
                    pk,
                    wk_sb[:, wc, fb * 128 : (fb + 1) * 128],
                    xt[:, xc, :],
                    start=(i == 0),
                    stop=(i == len(SPLIT3) - 1),
                )

            # order pk/pq so adjacent emissions hit alternating PSUM banks
            # once merged with the pv stream: [pk*16, (pk,pq)*8]
            stream = [lambda i=i: mm_pk(i) for i in range(16)]
            for i in range(8):
                stream.append(lambda i=i: mm_pk(16 + i))
                stream.append(lambda i=i: mm_pq(i))

            def finish(qT=qT, kT=kT, pq=pq, pk=pk, fb=fb):
                nc.scalar.sign(kT[:, fb * TT : (fb + 1) * TT], pk)
                nc.scalar.copy(qT[:, fb * TT : (fb + 1) * TT], pq)

            return stream, finish

        # prologue: x(0) + full projection of tile 0 (fb pairs interleaved
        # so adjacent matmuls target different PSUM banks)
        dma_x(0, 0, 16)
        wv_sb = const.tile([128, 16, 512], f16)
        for kc in range(16):
            nc.sync.dma_start(wv_sb[:, kc, :], wv[kc])
        for fbp in range(2):
            s_a, f_a = proj_qk(0, 2 * fbp)
            s_b, f_b = proj_qk(0, 2 * fbp + 1)
            for a, b in zip(s_a, s_b):
                a()
                b()
            f_a()
            f_b()
        dma_x(1, 0, 16)

        # remaining constants (deferred so they queue behind the hot DMAs)
        wo_sb = const.tile([128, 4, 1024], f16)
        nc.sync.dma_start(wo_sb, wo.rearrange("jc p i -> p jc i"))
        inv_sb = const.tile([128, T], f32)
        nc.sync.dma_start(inv_sb, invtot)
        mask_sb = const.tile([128, 512], f32)
        nc.sync.dma_start(mask_sb, mask2)
        id_sb = const.tile([128, 128], f16)
        nc.sync.dma_start(id_sb, ident)

        # running state S[kf, df] fp16 (exact ints), 2 heads stacked per tile
        s_cur = spool.tile([128, 256], f16, tag="S", name="s_init")
        nc.vector.memset(s_cur, 0.0)

        prev_osb = None

        def oproj_bigs(m, osb_m):
            """Return N=512 matmul thunks + finisher for chunk m's o_proj."""
            ysb = ypool.tile([128, 1024], f32, tag="ysb")
            yps = [
                pspool.tile([128, 512], f32, tag="work", name=f"yp{icol}")
                for icol in range(2)
            ]
            thunks = []
            for hp in range(4):
                for icol in range(2):
                    def mm_y(hp=hp, icol=icol):
                        nc.tensor.matmul(
                            yps[icol],
                            osb_m[:, hp * 128 : (hp + 1) * 128],
                            wo_sb[:, hp, icol * 512 : (icol + 1) * 512],
                            start=(hp == 0),
                            stop=(hp == 3),
                        )
                    thunks.append(mm_y)

            def finish():
                for icol in range(2):
                    nc.scalar.copy(ysb[:, icol * 512 : (icol + 1) * 512], yps[icol])
                nc.sync.dma_start(ypart[m * CH : (m + 1) * CH, :], ysb)

            return thunks, finish

        def emit_oproj(m, osb_m):
            thunks, finish = oproj_bigs(m, osb_m)
            for t in thunks:
                t()
            finish()

        pend = None  # (m, vtok, ktok, sts) of the previous chunk

        def attn_smalls(pend):
            """ot/ckv matmul thunks for the pending chunk (deps all ready)."""
            m, vtok, ktok, sts = pend
            tt_, ci_ = m // 4, m % 4
            qT_, _ = qk_tiles[tt_]
            c0_ = ci_ * CH
            ckv = pspool.tile([128, 256], f32, tag="work", name="ckv")
            ot = pspool.tile([128, 512], f32, tag="work", name="ot")
            thunks = []
            for h in range(8):
                bp = (h % 2) * 64
                hp = h // 2
                st = sts[h % 2]
                soff = (h // 4) * 256 + ((h // 2) % 2) * 128

                def mm_ic(h=h, bp=bp, hp=hp, st=st, soff=soff):
                    nc.tensor.matmul(
                        ot[bp : bp + 64, hp * 128 : hp * 128 + 128],
                        vtok[:, h * 64 : h * 64 + 64],
                        st[:, soff : soff + 128],
                        start=True,
                        stop=(m == 0),
                        tile_position=(0, bp),
                    )
                    if m > 0:
                        nc.tensor.matmul(
                            ot[bp : bp + 64, hp * 128 : hp * 128 + 128],
                            s_cur[bp : bp + 64, hp * 64 : hp * 64 + 64],
                            qT_[bp : bp + 64, hp * TT + c0_ : hp * TT + c0_ + CH],
                            start=False,
                            stop=True,
                            tile_position=(bp, bp),
                        )

                def mm_ckv(h=h, bp=bp):
                    nc.tensor.matmul(
                        ckv[bp : bp + 64, (h // 2) * 64 : (h // 2) * 64 + 64],
                        ktok[:, h * 64 : h * 64 + 64],
                        vtok[:, h * 64 : h * 64 + 64],
                        start=True,
                        stop=True,
                        tile_position=(0, bp),
                    )

                thunks.append(mm_ic)
                thunks.append(mm_ckv)
            return thunks, ckv, ot

        for n in range(NCH):
            tt, ci = n // 4, n % 4
            qT, kT = qk_tiles[tt]
            xt = xs_tiles[tt]
            c0 = ci * CH
            dma_x(tt + 2, ci * 4, ci * 4 + 4)

            # --- small-matmul work: previous chunk's ot/ckv, then this
            # chunk's scoresT + k transposes. scs2[0] holds row-group-0
            # heads (0,2,4,6), scs2[1] row-group-64 heads: same-bank tiles
            # share a PE row-group (concurrent different-row matmuls on one
            # bank collide fatally), banks alternate for concurrency. ---
            smalls = []
            if pend is not None:
                a_thunks, ckv_t, ot_t = attn_smalls(pend)
                smalls += a_thunks
            scs2 = [
                pspool.tile([128, 512], f32, tag="work", name=f"scs{i}")
                for i in range(2)
            ]
            ktp = pspool.tile([128, 512], f16, tag="work", name="ktp")

            def mm_score(h):
                bp = (h % 2) * 64
                hp = h // 2
                soff = (h // 4) * 256 + ((h // 2) % 2) * 128
                sl = slice(hp * TT + c0, hp * TT + c0 + CH)
                nc.tensor.matmul(
                    scs2[h % 2][:, soff : soff + 128],
                    kT[bp : bp + 64, sl],
                    qT[bp : bp + 64, sl],
                    start=True,
                    stop=True,
                )

            def mm_ktp(hp):
                sl = slice(hp * TT + c0, hp * TT + c0 + CH)
                nc.tensor.transpose(
                    ktp[:, hp * 128 : (hp + 1) * 128], kT[:, sl], id_sb
                )

            for h in range(4):
                smalls.append(lambda h=h: mm_score(h))
            for hp in range(4):
                smalls.append(lambda hp=hp: mm_score(4 + hp))
                smalls.append(lambda hp=hp: mm_ktp(hp))
            n_asm = len(smalls) - 12  # count of A-part smalls

            # --- N=512 stream: v proj (this chunk) + q/k proj (next tile)
            # + o_proj of the previous chunk, PSUM banks alternating ---
            pv = pspool.tile([128, 512], f32, tag="work", name="pv")

            def mm_pv(i):
                wc, xc = SPLIT3[i]
                nc.tensor.matmul(
                    pv,
                    xt[:, xc, c0 : c0 + CH],
                    wv_sb[:, wc, :],
                    start=(i == 0),
                    stop=(i == len(SPLIT3) - 1),
                )

            qk_stream, qk_finish = proj_qk(tt + 1, ci)
            bigs = []
            for i in range(max(24, len(qk_stream))):
                if i < 24:
                    bigs.append(lambda i=i: mm_pv(i))
                if i < len(qk_stream):
                    bigs.append(qk_stream[i])
            if prev_osb is not None:
                y_thunks, y_finish = oproj_bigs(n - 2, prev_osb)
                bigs += y_thunks
            else:
                y_finish = None

            # --- weave: pace smalls through the big stream ---
            nb = len(bigs)
            ns = len(smalls)
            bi = 0
            for si in range(ns):
                smalls[si]()
                if si == n_asm - 1 and pend is not None:
                    # previous chunk's state update + out scale (DVE)
                    m = pend[0]
                    s_new = spool.tile([128, 256], f16, tag="S", name="s_new")
                    nc.vector.tensor_add(s_new, s_cur, ckv_t)
                    s_cur = s_new
                    osb = opool.tile([128, 512], f16, tag="osb")
                    for hp in range(4):
                        nc.vector.tensor_mul(
                            osb[:, hp * 128 : (hp + 1) * 128],
                            ot_t[:, hp * 128 : (hp + 1) * 128],
                            inv_sb[:, m * CH : m * CH + CH],
                        )
                    next_osb = osb
                tgt = (si + 1) * nb // ns
                while bi < tgt:
                    bigs[bi]()
                    bi += 1
            while bi < nb:
                bigs[bi]()
                bi += 1
            qk_finish()
            if y_finish is not None:
                y_finish()
            vtok = tokpool.tile([128, 512], f16, tag="vtok")
            nc.scalar.sign(vtok, pv)
            sts = []
            for i in range(2):
                st = stpool.tile([128, 512], f16, tag="st")
                nc.vector.tensor_mul(st, scs2[i], mask_sb)
                sts.append(st)
            ktok = tokpool.tile([128, 512], f16, tag="ktok")
            nc.vector.tensor_copy(ktok, ktp)

            prev_osb = next_osb if pend is not None else None
            pend = (n, vtok, ktok, sts)

        # --- drain the last chunk: ot/ckv, state, scale, o_proj ---
        a_thunks, ckv_t, ot_t = attn_smalls(pend)
        if prev_osb is not None:
            y_thunks, y_finish = oproj_bigs(NCH - 2, prev_osb)
        for i, t in enumerate(a_thunks):
            t()
            if prev_osb is not None and i < len(y_thunks):
                y_thunks[i]()
        if prev_osb is not None:
            for t in y_thunks[len(a_thunks):]:
                t()
            y_finish()
        m = pend[0]
        s_new = spool.tile([128, 256], f16, tag="S", name="s_fin")
        nc.vector.tensor_add(s_new, s_cur, ckv_t)
        s_cur = s_new
        osb = opool.tile([128, 512], f16, tag="osb_fin")
        for hp in range(4):
            nc.vector.tensor_mul(
                osb[:, hp * 128 : (hp + 1) * 128],
                ot_t[:, hp * 128 : (hp + 1) * 128],
                inv_sb[:, m * CH : m * CH + CH],
            )
        emit_oproj(NCH - 1, osb)

        # --- final matrix (exact integer sums) ---
        fsb = opool.tile([128, 256], f32, tag="fsb")
        nc.vector.tensor_copy(fsb, s_cur)
        for h in range(8):
            nc.sync.dma_start(
                fmat[h],
                fsb[(h % 2) * 64 : (h % 2) * 64 + 64, (h // 2) * 64 : (h // 2) * 64 + 64],
            )

    nc.compile()
    return nc


def _split16(a):
    """fp16 double-double split along axis 0-stacking: returns (hi, lo)."""
    hi = a.astype(np.float16)
    lo = (a - hi.astype(np.float32)).astype(np.float16)
    return hi, lo


def _host_inputs(x, W_qkv, W_o):
    f32 = np.float32
    tvec = np.arange(1, T + 1, dtype=np.float64)
    inv = (1.0 / (8.0 * tvec)).astype(f32)
    invtot = np.ascontiguousarray(np.broadcast_to(inv[None, :], (128, T)))
    jj = np.arange(128)
    maskT = (jj[:, None] <= jj[None, :]).astype(f32)
    mask2 = np.ascontiguousarray(np.tile(maskT, (1, 4)))
    ident = np.eye(128, dtype=np.float16)

    Wq3 = np.asarray(W_qkv, dtype=f32).reshape(3, 16, DH, D)
    in_maps = []
    for core in range(8):
        b, g = core // 2, core % 2
        xT = np.asarray(x[b], dtype=f32).T  # [D, T]
        x1, x2 = _split16(xT)
        xs = np.ascontiguousarray(
            np.concatenate([x1, x2], axis=0).reshape(16, 128, T)
        )
        # weight column blocks for this head group, [D, 512] each
        wqc = Wq3[0, g * 8 : (g + 1) * 8].reshape(512, D).T
        wkc = Wq3[1, g * 8 : (g + 1) * 8].reshape(512, D).T
        wvc = Wq3[2, g * 8 : (g + 1) * 8].reshape(512, D).T
        wq = np.ascontiguousarray(wqc.astype(np.float16).reshape(8, 128, 512))
        wk1, wk2 = _split16(wkc)
        wk = np.ascontiguousarray(np.concatenate([wk1, wk2], axis=0).reshape(16, 128, 512))
        wv1, wv2 = _split16(wvc)
        wv = np.ascontiguousarray(np.concatenate([wv1, wv2], axis=0).reshape(16, 128, 512))
        wo = np.ascontiguousarray(
            np.asarray(W_o, dtype=f32)[:, g * 512 : (g + 1) * 512].T.astype(np.float16).reshape(4, 128, D)
        )
        in_maps.append(
            {
                "xs": xs,
                "wq": wq,
                "wk": wk,
                "wv": wv,
                "wo": wo,
                "invtot": invtot,
                "mask2": mask2,
                "ident": ident,
            }
        )
    return in_maps


def kernel(x, W_qkv, W_o, trace=False):
    from concourse import bass_utils

    nc = _build()
    in_maps = _host_inputs(x, W_qkv, W_o)
    res = bass_utils.run_bass_kernel_spmd(
        nc, in_maps, core_ids=list(range(8)), trace=trace
    )
    results = res.results

    f32 = np.float32
    y = np.empty((4, T, D), dtype=f32)
    fm = np.empty((4, 16, DH, DH), dtype=f32)
    for core in range(8):
        b, g = core // 2, core % 2
        if g == 0:
            y[b] = results[core]["ypart"]
        else:
            y[b] += results[core]["ypart"]
        fm[b, g * 8 : (g + 1) * 8] = results[core]["fmat"]
    fc = np.full((4, 16, 1, 1), float(T), dtype=f32)
    if trace:
        kernel._last_result = res
    return (y, fm, fc)


# revision 20
# speedup vs baseline: 1.1111x; 1.1111x over previous
"""Binary associative memory (causal linear attention with binarized k/v).

Self-contained Trainium2 Bass kernel.

Math: the reference's chunked prefix recurrence telescopes to exact causal
linear attention:
    out[t] = (1/(8*(t+1))) * sum_{s<=t} (q[t].k[s]) v[s],   k,v = sign(qkv)
    y      = out @ W_o.T   (summed over head features)
    final_matrix[b,h] = sum_t k[t] (x) v[t]   (exact integers)
so we are free to re-chunk at 128 tokens (partition width).

Sharding: 8 cores = 4 batches x 2 head-groups (8 heads each).

Precision: k/v binarize must match the fp32 reference's signs, so the k/v
projections use an fp16 double-double trick: x ~= x1 + x2 and W ~= w1 + w2
(fp16 splits, prepared on host) stacked along the contraction dim, so one
K=2048 accumulation computes (x1+x2)@(w1+w2) with ~1e-7 relative error at
full PE speed. q (continuous) uses a single fp16 term; o_proj runs fp16.

Layouts per core:
  - q,k projected feature-on-partition ([feat, tok]) for scoresT/crossT
  - v projected token-on-partition directly ([tok, vfeat]) for intra/ckv
  - k transposed per chunk on the PE (with identity) for ckv
  - state S[kf, df] fp16 (exact small integers), 2 heads per 128 partitions
"""

import functools

import numpy as np

T = 4096
D = 1024
HLOC = 8  # heads per core
DH = 64
CH = 128  # chunk size
NCH = T // CH  # 32
TT = 512  # projection token tile
NTT = T // TT  # 8

PAIRS = [(0, 2), (1, 3), (4, 6), (5, 7)]  # scores PSUM-tile pairs (same row-group)
# 3-term fp16 split of x@W: x1w1 + x2w1 + x1w2 (chunk indices into [hi(8), lo(8)])
SPLIT3 = [(wc, xc) for wc, xc in
          [(c, c) for c in range(8)] + [(c, c + 8) for c in range(8)] + [(c + 8, c) for c in range(8)]]


@functools.lru_cache(maxsize=1)
def _build():
    from contextlib import ExitStack

    import concourse.bacc as bacc
    import concourse.mybir as mybir
    import concourse.tile as tile

    f32 = mybir.dt.float32
    f16 = mybir.dt.float16

    nc = bacc.Bacc("TRN2", target_bir_lowering=False, debug=False, num_devices=8)

    # x split [x1; x2], chunk-major: [16 kc][128][T]
    xs = nc.dram_tensor("xs", [16, 128, T], f16, kind="ExternalInput").ap()
    wq = nc.dram_tensor("wq", [8, 128, 512], f16, kind="ExternalInput").ap()
    wk = nc.dram_tensor("wk", [16, 128, 512], f16, kind="ExternalInput").ap()
    wv = nc.dram_tensor("wv", [16, 128, 512], f16, kind="ExternalInput").ap()
    wo = nc.dram_tensor("wo", [4, 128, 1024], f16, kind="ExternalInput").ap()
    invtot = nc.dram_tensor("invtot", [128, T], f32, kind="ExternalInput").ap()
    mask2 = nc.dram_tensor("mask2", [128, 512], f32, kind="ExternalInput").ap()
    ident = nc.dram_tensor("ident", [128, 128], f16, kind="ExternalInput").ap()
    ypart = nc.dram_tensor("ypart", [T, D], f32, kind="ExternalOutput").ap()
    fmat = nc.dram_tensor("fmat", [HLOC, DH, DH], f32, kind="ExternalOutput").ap()

    with tile.TileContext(nc) as tc, ExitStack() as ctx:
        const = ctx.enter_context(tc.tile_pool(name="const", bufs=1))
        xpool = ctx.enter_context(tc.tile_pool(name="xp", bufs=3))
        qkpool = ctx.enter_context(tc.tile_pool(name="qk", bufs=2))
        tokpool = ctx.enter_context(tc.tile_pool(name="tok", bufs=3))
        stpool = ctx.enter_context(tc.tile_pool(name="st", bufs=8))
        opool = ctx.enter_context(tc.tile_pool(name="op", bufs=3))
        ypool = ctx.enter_context(tc.tile_pool(name="yp", bufs=3))
        spool = ctx.enter_context(tc.tile_pool(name="sp", bufs=2))
        pspool = ctx.enter_context(tc.tile_pool(name="ps", bufs=8, space="PSUM"))

        # weights needed by the first projections: emit DMAs first
        wq_sb = const.tile([128, 8, 512], f16)
        for kc in range(8):
            nc.sync.dma_start(wq_sb[:, kc, :], wq[kc])
        wk_sb = const.tile([128, 16, 512], f16)
        for kc in range(16):
            nc.sync.dma_start(wk_sb[:, kc, :], wk[kc])

        xs_tiles = {}

        def dma_x(tt, kc_lo, kc_hi):
            if tt >= NTT:
                return
            if tt not in xs_tiles:
                xs_tiles[tt] = xpool.tile([128, 16, TT], f16, tag="xs", name="xs_sb")
            t0 = tt * TT
            for kc in range(kc_lo, kc_hi):
                nc.sync.dma_start(xs_tiles[tt][:, kc, :], xs[kc, :, t0 : t0 + TT])

        qk_tiles = {}

        def proj_qk(tt, fb):
            """Build projection emitters for q/k feature-block fb of tile tt.

            Returns (stream, finish): stream is a list of matmul thunks to
            interleave with other PSUM banks; finish emits the evacuations."""
            if tt >= NTT:
                return [], lambda: None
            if tt not in qk_tiles:
                qk_tiles[tt] = (
                    qkpool.tile([128, 4 * TT], f16, tag="qT", name="qT_sb"),
                    qkpool.tile([128, 4 * TT], f16, tag="kT", name="kT_sb"),
                )
            qT, kT = qk_tiles[tt]
            xt = xs_tiles[tt]
            pq = pspool.tile([128, 512], f32, tag="work", name="pq")
            pk = pspool.tile([128, 512], f32, tag="work", name="pk")

            def mm_pq(kc, pq=pq, xt=xt, fb=fb):
                nc.tensor.matmul(
                    pq,
                    wq_sb[:, kc, fb * 128 : (fb + 1) * 128],
                    xt[:, kc, :],
                    start=(kc == 0),
                    stop=(kc == 7),
                )

            def mm_pk(i, pk=pk, xt=xt, fb=fb):
                wc, xc = SPLIT3[i]
                nc.tensor.matmul(
                    pk,
                    wk_sb[:, wc, fb * 128 : (fb + 1) * 128],
                    xt[:, xc, :],
                    start=(i == 0),
                    stop=(i == len(SPLIT3) - 1),
                )

            # order pk/pq so adjacent emissions hit alternating PSUM banks
            # once merged with the pv stream: [pk*16, (pk,pq)*8]
            stream = [lambda i=i: mm_pk(i) for i in range(16)]
            for i in range(8):
                stream.append(lambda i=i: mm_pk(16 + i))
                stream.append(lambda i=i: mm_pq(i))

            def finish(qT=qT, kT=kT, pq=pq, pk=pk, fb=fb):
                nc.scalar.sign(kT[:, fb * TT : (fb + 1) * TT], pk)
                nc.scalar.copy(qT[:, fb * TT : (fb + 1) * TT], pq)

            return stream, finish

        # prologue: x(0) + full projection of tile 0 (fb pairs interleaved
        # so adjacent matmuls target different PSUM banks)
        dma_x(0, 0, 16)
        wv_sb = const.tile([128, 16, 512], f16)
        for kc in range(16):
            nc.sync.dma_start(wv_sb[:, kc, :], wv[kc])
        for fbp in range(2):
            s_a, f_a = proj_qk(0, 2 * fbp)
            s_b, f_b = proj_qk(0, 2 * fbp + 1)
            for a, b in zip(s_a, s_b):
                a()
                b()
            f_a()
            f_b()
        dma_x(1, 0, 16)

        # remaining constants (deferred so they queue behind the hot DMAs)
        wo_sb = const.tile([128, 4, 1024], f16)
        nc.sync.dma_start(wo_sb, wo.rearrange("jc p i -> p jc i"))
        inv_sb = const.tile([128, T], f32)
        nc.sync.dma_start(inv_sb, invtot)
        mask_sb = const.tile([128, 512], f32)
        nc.sync.dma_start(mask_sb, mask2)
        id_sb = const.tile([128, 128], f16)
        nc.sync.dma_start(id_sb, ident)

        # running state S[kf, df] fp16 (exact ints), 2 heads stacked per tile
        s_cur = spool.tile([128, 256], f16, tag="S", name="s_init")
        nc.vector.memset(s_cur, 0.0)

        prev_osb = None

        def oproj_bigs(m, osb_m):
            """Return N=512 matmul thunks + finisher for chunk m's o_proj."""
            ysb = ypool.tile([128, 1024], f32, tag="ysb")
            yps = [
                pspool.tile([128, 512], f32, tag="work", name=f"yp{icol}")
                for icol in range(2)
            ]
            thunks = []
            for hp in range(4):
                for icol in range(2):
                    def mm_y(hp=hp, icol=icol):
                        nc.tensor.matmul(
                            yps[icol],
                            osb_m[:, hp * 128 : (hp + 1) * 128],
                            wo_sb[:, hp, icol * 512 : (icol + 1) * 512],
                            start=(hp == 0),
                            stop=(hp == 3),
                        )
                    thunks.append(mm_y)

            def finish():
                for icol in range(2):
                    nc.scalar.copy(ysb[:, icol * 512 : (icol + 1) * 512], yps[icol])
                nc.sync.dma_start(ypart[m * CH : (m + 1) * CH, :], ysb)

            return thunks, finish

        def emit_oproj(m, osb_m):
            thunks, finish = oproj_bigs(m, osb_m)
            for t in thunks:
                t()
            finish()

        prev_osb = None
        for n in range(NCH):
            tt, ci = n // 4, n % 4
            qT, kT = qk_tiles[tt]
            xt = xs_tiles[tt]
            c0 = ci * CH
            dma_x(tt + 2, ci * 4, ci * 4 + 4)

            # --- scoresT[j, i] + k transposes. scs2[0] holds row-group-0
            # heads (0,2,4,6), scs2[1] the row-group-64 heads: same-bank
            # matmuls must share a PE row-group (concurrent different-row
            # matmuls on one bank collide fatally); the two banks overlap. ---
            scs2 = [
                pspool.tile([128, 512], f32, tag="work", name=f"scs{i}")
                for i in range(2)
            ]
            ktp = pspool.tile([128, 512], f16, tag="work", name="ktp")

            def mm_score(h):
                bp = (h % 2) * 64
                hp = h // 2
                soff = (h // 4) * 256 + ((h // 2) % 2) * 128
                sl = slice(hp * TT + c0, hp * TT + c0 + CH)
                nc.tensor.matmul(
                    scs2[h % 2][:, soff : soff + 128],
                    kT[bp : bp + 64, sl],
                    qT[bp : bp + 64, sl],
                    start=True,
                    stop=True,
                )

            for h in range(4):
                mm_score(h)
            for hp in range(4):
                mm_score(4 + hp)
                sl = slice(hp * TT + c0, hp * TT + c0 + CH)
                nc.tensor.transpose(
                    ktp[:, hp * 128 : (hp + 1) * 128], kT[:, sl], id_sb
                )
            sts = []
            for i in range(2):
                st = stpool.tile([128, 512], f16, tag="st")
                nc.vector.tensor_mul(st, scs2[i], mask_sb)
                sts.append(st)
            ktok = tokpool.tile([128, 512], f16, tag="ktok")
            nc.vector.tensor_copy(ktok, ktp)

            # --- o_proj of the previous chunk: N=512 work that keeps the
            # array busy right after the smalls ---
            if prev_osb is not None:
                emit_oproj(n - 1, prev_osb)
                prev_osb = None

            # --- merged projection stream: v (this chunk) + q/k of the
            # next tile, alternating PSUM banks to hide drain bubbles ---
            pv = pspool.tile([128, 512], f32, tag="work", name="pv")

            def mm_pv(i):
                wc, xc = SPLIT3[i]
                nc.tensor.matmul(
                    pv,
                    xt[:, xc, c0 : c0 + CH],
                    wv_sb[:, wc, :],
                    start=(i == 0),
                    stop=(i == len(SPLIT3) - 1),
                )

            qk_stream, qk_finish = proj_qk(tt + 1, ci)
            for i in range(max(24, len(qk_stream))):
                if i < 24:
                    mm_pv(i)
                if i < len(qk_stream):
                    qk_stream[i]()
            qk_finish()
            vtok = tokpool.tile([128, 512], f16, tag="vtok")
            nc.scalar.sign(vtok, pv)

            # --- ckv[kf, df] + outT = intraT + crossT, interleaved ---
            ckv = pspool.tile([128, 256], f32, tag="work", name="ckv")
            ot = pspool.tile([128, 512], f32, tag="work", name="ot")
            for h in range(8):
                bp = (h % 2) * 64
                hp = h // 2
                soff = (h // 4) * 256 + ((h // 2) % 2) * 128
                nc.tensor.matmul(
                    ot[bp : bp + 64, hp * 128 : hp * 128 + 128],
                    vtok[:, h * 64 : h * 64 + 64],
                    sts[h % 2][:, soff : soff + 128],
                    start=True,
                    stop=(n == 0),
                    tile_position=(0, bp),
                )
                if n > 0:
                    nc.tensor.matmul(
                        ot[bp : bp + 64, hp * 128 : hp * 128 + 128],
                        s_cur[bp : bp + 64, hp * 64 : hp * 64 + 64],
                        qT[bp : bp + 64, hp * TT + c0 : hp * TT + c0 + CH],
                        start=False,
                        stop=True,
                        tile_position=(bp, bp),
                    )
                nc.tensor.matmul(
                    ckv[bp : bp + 64, (h // 2) * 64 : (h // 2) * 64 + 64],
                    ktok[:, h * 64 : h * 64 + 64],
                    vtok[:, h * 64 : h * 64 + 64],
                    start=True,
                    stop=True,
                    tile_position=(0, bp),
                )

            # --- state update + out scale (DVE) ---
            s_new = spool.tile([128, 256], f16, tag="S", name="s_new")
            nc.vector.tensor_add(s_new, s_cur, ckv)
            s_cur = s_new
            osb = opool.tile([128, 512], f16, tag="osb")
            for hp in range(4):
                nc.vector.tensor_mul(
                    osb[:, hp * 128 : (hp + 1) * 128],
                    ot[:, hp * 128 : (hp + 1) * 128],
                    inv_sb[:, n * CH : n * CH + CH],
                )
            prev_osb = osb

        emit_oproj(NCH - 1, prev_osb)

        # --- final matrix (exact integer sums) ---
        fsb = opool.tile([128, 256], f32, tag="fsb")
        nc.vector.tensor_copy(fsb, s_cur)
        for h in range(8):
            nc.sync.dma_start(
                fmat[h],
                fsb[(h % 2) * 64 : (h % 2) * 64 + 64, (h // 2) * 64 : (h // 2) * 64 + 64],
            )

    nc.compile()
    return nc


def _split16(a):
    """fp16 double-double split along axis 0-stacking: returns (hi, lo)."""
    hi = a.astype(np.float16)
    lo = (a - hi.astype(np.float32)).astype(np.float16)
    return hi, lo


def _host_inputs(x, W_qkv, W_o):
    f32 = np.float32
    tvec = np.arange(1, T + 1, dtype=np.float64)
    inv = (1.0 / (8.0 * tvec)).astype(f32)
    invtot = np.ascontiguousarray(np.broadcast_to(inv[None, :], (128, T)))
    jj = np.arange(128)
    maskT = (jj[:, None] <= jj[None, :]).astype(f32)
    mask2 = np.ascontiguousarray(np.tile(maskT, (1, 4)))
    ident = np.eye(128, dtype=np.float16)

    Wq3 = np.asarray(W_qkv, dtype=f32).reshape(3, 16, DH, D)
    in_maps = []
    for core in range(8):
        b, g = core // 2, core % 2
        xT = np.asarray(x[b], dtype=f32).T  # [D, T]
        x1, x2 = _split16(xT)
        xs = np.ascontiguousarray(
            np.concatenate([x1, x2], axis=0).reshape(16, 128, T)
        )
        # weight column blocks for this head group, [D, 512] each
        wqc = Wq3[0, g * 8 : (g + 1) * 8].reshape(512, D).T
        wkc = Wq3[1, g * 8 : (g + 1) * 8].reshape(512, D).T
        wvc = Wq3[2, g * 8 : (g + 1) * 8].reshape(512, D).T
        wq = np.ascontiguousarray(wqc.astype(np.float16).reshape(8, 128, 512))
        wk1, wk2 = _split16(wkc)
        wk = np.ascontiguousarray(np.concatenate([wk1, wk2], axis=0).reshape(16, 128, 512))
        wv1, wv2 = _split16(wvc)
        wv = np.ascontiguousarray(np.concatenate([wv1, wv2], axis=0).reshape(16, 128, 512))
        wo = np.ascontiguousarray(
            np.asarray(W_o, dtype=f32)[:, g * 512 : (g + 1) * 512].T.astype(np.float16).reshape(4, 128, D)
        )
        in_maps.append(
            {
                "xs": xs,
                "wq": wq,
                "wk": wk,
                "wv": wv,
                "wo": wo,
                "invtot": invtot,
                "mask2": mask2,
                "ident": ident,
            }
        )
    return in_maps


def kernel(x, W_qkv, W_o, trace=False):
    from concourse import bass_utils

    nc = _build()
    in_maps = _host_inputs(x, W_qkv, W_o)
    res = bass_utils.run_bass_kernel_spmd(
        nc, in_maps, core_ids=list(range(8)), trace=trace
    )
    results = res.results

    f32 = np.float32
    y = np.empty((4, T, D), dtype=f32)
    fm = np.empty((4, 16, DH, DH), dtype=f32)
    for core in range(8):
        b, g = core // 2, core % 2
        if g == 0:
            y[b] = results[core]["ypart"]
        else:
            y[b] += results[core]["ypart"]
        fm[b, g * 8 : (g + 1) * 8] = results[core]["fmat"]
    fc = np.full((4, 16, 1, 1), float(T), dtype=f32)
    if trace:
        kernel._last_result = res
    return (y, fm, fc)


# revision 21
# speedup vs baseline: 1.1132x; 1.0019x over previous
"""Binary associative memory (causal linear attention with binarized k/v).

Self-contained Trainium2 Bass kernel.

Math: the reference's chunked prefix recurrence telescopes to exact causal
linear attention:
    out[t] = (1/(8*(t+1))) * sum_{s<=t} (q[t].k[s]) v[s],   k,v = sign(qkv)
    y      = out @ W_o.T   (summed over head features)
    final_matrix[b,h] = sum_t k[t] (x) v[t]   (exact integers)
so we are free to re-chunk at 128 tokens (partition width).

Sharding: 8 cores = 4 batches x 2 head-groups (8 heads each).

Precision: k/v binarize must match the fp32 reference's signs, so the k/v
projections use an fp16 double-double trick: x ~= x1 + x2 and W ~= w1 + w2
(fp16 splits, prepared on host) stacked along the contraction dim, so one
K=2048 accumulation computes (x1+x2)@(w1+w2) with ~1e-7 relative error at
full PE speed. q (continuous) uses a single fp16 term; o_proj runs fp16.

Layouts per core:
  - q,k projected feature-on-partition ([feat, tok]) for scoresT/crossT
  - v projected token-on-partition directly ([tok, vfeat]) for intra/ckv
  - k transposed per chunk on the PE (with identity) for ckv
  - state S[kf, df] fp16 (exact small integers), 2 heads per 128 partitions
"""

import functools

import numpy as np

T = 4096
D = 1024
HLOC = 8  # heads per core
DH = 64
CH = 128  # chunk size
NCH = T // CH  # 32
TT = 512  # projection token tile
NTT = T // TT  # 8

PAIRS = [(0, 2), (1, 3), (4, 6), (5, 7)]  # scores PSUM-tile pairs (same row-group)
# 3-term fp16 split of x@W: x1w1 + x2w1 + x1w2 (chunk indices into [hi(8), lo(8)])
SPLIT3 = [(wc, xc) for wc, xc in
          [(c, c) for c in range(8)] + [(c, c + 8) for c in range(8)] + [(c + 8, c) for c in range(8)]]


@functools.lru_cache(maxsize=1)
def _build():
    from contextlib import ExitStack

    import concourse.bacc as bacc
    import concourse.mybir as mybir
    import concourse.tile as tile

    f32 = mybir.dt.float32
    f16 = mybir.dt.float16

    nc = bacc.Bacc("TRN2", target_bir_lowering=False, debug=False, num_devices=8)

    # x split [x1; x2], chunk-major: [16 kc][128][T]
    xs = nc.dram_tensor("xs", [16, 128, T], f16, kind="ExternalInput").ap()
    wq = nc.dram_tensor("wq", [8, 128, 512], f16, kind="ExternalInput").ap()
    wk = nc.dram_tensor("wk", [16, 128, 512], f16, kind="ExternalInput").ap()
    wv = nc.dram_tensor("wv", [16, 128, 512], f16, kind="ExternalInput").ap()
    wo = nc.dram_tensor("wo", [4, 128, 1024], f16, kind="ExternalInput").ap()
    invtot = nc.dram_tensor("invtot", [128, T], f32, kind="ExternalInput").ap()
    mask2 = nc.dram_tensor("mask2", [128, 512], f32, kind="ExternalInput").ap()
    ident = nc.dram_tensor("ident", [128, 128], f16, kind="ExternalInput").ap()
    ypart = nc.dram_tensor("ypart", [T, D], f32, kind="ExternalOutput").ap()
    fmat = nc.dram_tensor("fmat", [HLOC, DH, DH], f32, kind="ExternalOutput").ap()

    with tile.TileContext(nc) as tc, ExitStack() as ctx:
        const = ctx.enter_context(tc.tile_pool(name="const", bufs=1))
        xpool = ctx.enter_context(tc.tile_pool(name="xp", bufs=3))
        qkpool = ctx.enter_context(tc.tile_pool(name="qk", bufs=2))
        tokpool = ctx.enter_context(tc.tile_pool(name="tok", bufs=3))
        stpool = ctx.enter_context(tc.tile_pool(name="st", bufs=8))
        opool = ctx.enter_context(tc.tile_pool(name="op", bufs=3))
        ypool = ctx.enter_context(tc.tile_pool(name="yp", bufs=3))
        spool = ctx.enter_context(tc.tile_pool(name="sp", bufs=2))
        pspool = ctx.enter_context(tc.tile_pool(name="ps", bufs=8, space="PSUM"))

        # weights needed by the first projections: emit DMAs first
        # (wk feeds the leading pk matmuls, so it goes before wq)
        wk_sb = const.tile([128, 16, 512], f16)
        for kc in range(16):
            nc.sync.dma_start(wk_sb[:, kc, :], wk[kc])
        wq_sb = const.tile([128, 8, 512], f16)
        for kc in range(8):
            nc.sync.dma_start(wq_sb[:, kc, :], wq[kc])

        xs_tiles = {}

        def dma_x(tt, kc_lo, kc_hi):
            if tt >= NTT:
                return
            if tt not in xs_tiles:
                xs_tiles[tt] = xpool.tile([128, 16, TT], f16, tag="xs", name="xs_sb")
            t0 = tt * TT
            for kc in range(kc_lo, kc_hi):
                nc.sync.dma_start(xs_tiles[tt][:, kc, :], xs[kc, :, t0 : t0 + TT])

        qk_tiles = {}

        def proj_qk(tt, fb):
            """Build projection emitters for q/k feature-block fb of tile tt.

            Returns (stream, finish): stream is a list of matmul thunks to
            interleave with other PSUM banks; finish emits the evacuations."""
            if tt >= NTT:
                return [], lambda: None
            if tt not in qk_tiles:
                qk_tiles[tt] = (
                    qkpool.tile([128, 4 * TT], f16, tag="qT", name="qT_sb"),
                    qkpool.tile([128, 4 * TT], f16, tag="kT", name="kT_sb"),
                )
            qT, kT = qk_tiles[tt]
            xt = xs_tiles[tt]
            pq = pspool.tile([128, 512], f32, tag="work", name="pq")
            pk = pspool.tile([128, 512], f32, tag="work", name="pk")

            def mm_pq(kc, pq=pq, xt=xt, fb=fb):
                nc.tensor.matmul(
                    pq,
                    wq_sb[:, kc, fb * 128 : (fb + 1) * 128],
                    xt[:, kc, :],
                    start=(kc == 0),
                    stop=(kc == 7),
                )

            def mm_pk(i, pk=pk, xt=xt, fb=fb):
                wc, xc = SPLIT3[i]
                nc.tensor.matmul(
                    pk,
                    wk_sb[:, wc, fb * 128 : (fb + 1) * 128],
                    xt[:, xc, :],
                    start=(i == 0),
                    stop=(i == len(SPLIT3) - 1),
                )

            # order pk/pq so adjacent emissions hit alternating PSUM banks
            # once merged with the pv stream: [pk*16, (pk,pq)*8]
            stream = [lambda i=i: mm_pk(i) for i in range(16)]
            for i in range(8):
                stream.append(lambda i=i: mm_pk(16 + i))
                stream.append(lambda i=i: mm_pq(i))

            def finish(qT=qT, kT=kT, pq=pq, pk=pk, fb=fb):
                nc.scalar.sign(kT[:, fb * TT : (fb + 1) * TT], pk)
                nc.scalar.copy(qT[:, fb * TT : (fb + 1) * TT], pq)

            return stream, finish

        # prologue: x(0) + full projection of tile 0 (fb pairs interleaved
        # so adjacent matmuls target different PSUM banks)
        dma_x(0, 0, 16)
        wv_sb = const.tile([128, 16, 512], f16)
        for kc in range(16):
            nc.sync.dma_start(wv_sb[:, kc, :], wv[kc])
        for fbp in range(2):
            s_a, f_a = proj_qk(0, 2 * fbp)
            s_b, f_b = proj_qk(0, 2 * fbp + 1)
            for a, b in zip(s_a, s_b):
                a()
                b()
            f_a()
            f_b()
        dma_x(1, 0, 16)

        # remaining constants (deferred so they queue behind the hot DMAs)
        wo_sb = const.tile([128, 4, 1024], f16)
        nc.sync.dma_start(wo_sb, wo.rearrange("jc p i -> p jc i"))
        inv_sb = const.tile([128, T], f32)
        nc.sync.dma_start(inv_sb, invtot)
        mask_sb = const.tile([128, 512], f32)
        nc.sync.dma_start(mask_sb, mask2)
        id_sb = const.tile([128, 128], f16)
        nc.sync.dma_start(id_sb, ident)

        # running state S[kf, df] fp16 (exact ints), 2 heads stacked per tile
        s_cur = spool.tile([128, 256], f16, tag="S", name="s_init")
        nc.vector.memset(s_cur, 0.0)

        prev_osb = None

        def oproj_bigs(m, osb_m):
            """Return N=512 matmul thunks + finisher for chunk m's o_proj."""
            ysb = ypool.tile([128, 1024], f32, tag="ysb")
            yps = [
                pspool.tile([128, 512], f32, tag="work", name=f"yp{icol}")
                for icol in range(2)
            ]
            thunks = []
            for hp in range(4):
                for icol in range(2):
                    def mm_y(hp=hp, icol=icol):
                        nc.tensor.matmul(
                            yps[icol],
                            osb_m[:, hp * 128 : (hp + 1) * 128],
                            wo_sb[:, hp, icol * 512 : (icol + 1) * 512],
                            start=(hp == 0),
                            stop=(hp == 3),
                        )
                    thunks.append(mm_y)

            def finish():
                for icol in range(2):
                    nc.scalar.copy(ysb[:, icol * 512 : (icol + 1) * 512], yps[icol])
                nc.sync.dma_start(ypart[m * CH : (m + 1) * CH, :], ysb)

            return thunks, finish

        def emit_oproj(m, osb_m):
            thunks, finish = oproj_bigs(m, osb_m)
            for t in thunks:
                t()
            finish()

        prev_osb = None
        for n in range(NCH):
            tt, ci = n // 4, n % 4
            qT, kT = qk_tiles[tt]
            xt = xs_tiles[tt]
            c0 = ci * CH
            dma_x(tt + 2, ci * 4, ci * 4 + 4)

            # --- scoresT[j, i] + k transposes. scs2[0] holds row-group-0
            # heads (0,2,4,6), scs2[1] the row-group-64 heads: same-bank
            # matmuls must share a PE row-group (concurrent different-row
            # matmuls on one bank collide fatally); the two banks overlap. ---
            scs2 = [
                pspool.tile([128, 512], f32, tag="work", name=f"scs{i}")
                for i in range(2)
            ]
            ktp = pspool.tile([128, 512], f16, tag="work", name="ktp")

            def mm_score(h):
                bp = (h % 2) * 64
                hp = h // 2
                soff = (h // 4) * 256 + ((h // 2) % 2) * 128
                sl = slice(hp * TT + c0, hp * TT + c0 + CH)
                nc.tensor.matmul(
                    scs2[h % 2][:, soff : soff + 128],
                    kT[bp : bp + 64, sl],
                    qT[bp : bp + 64, sl],
                    start=True,
                    stop=True,
                )

            for h in range(4):
                mm_score(h)
            for hp in range(4):
                mm_score(4 + hp)
                sl = slice(hp * TT + c0, hp * TT + c0 + CH)
                nc.tensor.transpose(
                    ktp[:, hp * 128 : (hp + 1) * 128], kT[:, sl], id_sb
                )
            sts = []
            for i in range(2):
                st = stpool.tile([128, 512], f16, tag="st")
                nc.vector.tensor_mul(st, scs2[i], mask_sb)
                sts.append(st)
            ktok = tokpool.tile([128, 512], f16, tag="ktok")
            nc.vector.tensor_copy(ktok, ktp)

            # --- merged projection stream: v (this chunk) + q/k of the
            # next tile, alternating PSUM banks to hide drain bubbles ---
            pv = pspool.tile([128, 512], f32, tag="work", name="pv")

            def mm_pv(i):
                wc, xc = SPLIT3[i]
                nc.tensor.matmul(
                    pv,
                    xt[:, xc, c0 : c0 + CH],
                    wv_sb[:, wc, :],
                    start=(i == 0),
                    stop=(i == len(SPLIT3) - 1),
                )

            qk_stream, qk_finish = proj_qk(tt + 1, ci)
            for i in range(max(24, len(qk_stream))):
                if i < 24:
                    mm_pv(i)
                if i < len(qk_stream):
                    qk_stream[i]()
            qk_finish()
            vtok = tokpool.tile([128, 512], f16, tag="vtok")
            nc.scalar.sign(vtok, pv)

            # --- ckv[kf, df] + outT = intraT + crossT, interleaved ---
            ckv = pspool.tile([128, 256], f32, tag="work", name="ckv")
            ot = pspool.tile([128, 512], f32, tag="work", name="ot")
            for h in range(8):
                bp = (h % 2) * 64
                hp = h // 2
                soff = (h // 4) * 256 + ((h // 2) % 2) * 128
                nc.tensor.matmul(
                    ot[bp : bp + 64, hp * 128 : hp * 128 + 128],
                    vtok[:, h * 64 : h * 64 + 64],
                    sts[h % 2][:, soff : soff + 128],
                    start=True,
                    stop=(n == 0),
                    tile_position=(0, bp),
                )
                if n > 0:
                    nc.tensor.matmul(
                        ot[bp : bp + 64, hp * 128 : hp * 128 + 128],
                        s_cur[bp : bp + 64, hp * 64 : hp * 64 + 64],
                        qT[bp : bp + 64, hp * TT + c0 : hp * TT + c0 + CH],
                        start=False,
                        stop=True,
                        tile_position=(bp, bp),
                    )
                nc.tensor.matmul(
                    ckv[bp : bp + 64, (h // 2) * 64 : (h // 2) * 64 + 64],
                    ktok[:, h * 64 : h * 64 + 64],
                    vtok[:, h * 64 : h * 64 + 64],
                    start=True,
                    stop=True,
                    tile_position=(0, bp),
                )

            # --- o_proj of the previous chunk: dense N=512 work splitting
            # the ckv/ot smalls from the next chunk's scores smalls, so no
            # HAM window sees a long low-duty stretch ---
            if prev_osb is not None:
                emit_oproj(n - 1, prev_osb)
                prev_osb = None

            # --- state update + out scale (DVE) ---
            s_new = spool.tile([128, 256], f16, tag="S", name="s_new")
            nc.vector.tensor_add(s_new, s_cur, ckv)
            s_cur = s_new
            osb = opool.tile([128, 512], f16, tag="osb")
            for hp in range(4):
                nc.vector.tensor_mul(
                    osb[:, hp * 128 : (hp + 1) * 128],
                    ot[:, hp * 128 : (hp + 1) * 128],
                    inv_sb[:, n * CH : n * CH + CH],
                )
            prev_osb = osb

        emit_oproj(NCH - 1, prev_osb)

        # --- final matrix (exact integer sums) ---
        fsb = opool.tile([128, 256], f32, tag="fsb")
        nc.vector.tensor_copy(fsb, s_cur)
        for h in range(8):
            nc.sync.dma_start(
                fmat[h],
                fsb[(h % 2) * 64 : (h % 2) * 64 + 64, (h // 2) * 64 : (h // 2) * 64 + 64],
            )

    nc.compile()
    return nc


def _split16(a):
    """fp16 double-double split along axis 0-stacking: returns (hi, lo)."""
    hi = a.astype(np.float16)
    lo = (a - hi.astype(np.float32)).astype(np.float16)
    return hi, lo


def _host_inputs(x, W_qkv, W_o):
    f32 = np.float32
    tvec = np.arange(1, T + 1, dtype=np.float64)
    inv = (1.0 / (8.0 * tvec)).astype(f32)
    invtot = np.ascontiguousarray(np.broadcast_to(inv[None, :], (128, T)))
    jj = np.arange(128)
    maskT = (jj[:, None] <= jj[None, :]).astype(f32)
    mask2 = np.ascontiguousarray(np.tile(maskT, (1, 4)))
    ident = np.eye(128, dtype=np.float16)

    Wq3 = np.asarray(W_qkv, dtype=f32).reshape(3, 16, DH, D)
    in_maps = []
    for core in range(8):
        b, g = core // 2, core % 2
        xT = np.asarray(x[b], dtype=f32).T  # [D, T]
        x1, x2 = _split16(xT)
        xs = np.ascontiguousarray(
            np.concatenate([x1, x2], axis=0).reshape(16, 128, T)
        )
        # weight column blocks for this head group, [D, 512] each
        wqc = Wq3[0, g * 8 : (g + 1) * 8].reshape(512, D).T
        wkc = Wq3[1, g * 8 : (g + 1) * 8].reshape(512, D).T
        wvc = Wq3[2, g * 8 : (g + 1) * 8].reshape(512, D).T
        wq = np.ascontiguousarray(wqc.astype(np.float16).reshape(8, 128, 512))
        wk1, wk2 = _split16(wkc)
        wk = np.ascontiguousarray(np.concatenate([wk1, wk2], axis=0).reshape(16, 128, 512))
        wv1, wv2 = _split16(wvc)
        wv = np.ascontiguousarray(np.concatenate([wv1, wv2], axis=0).reshape(16, 128, 512))
        wo = np.ascontiguousarray(
            np.asarray(W_o, dtype=f32)[:, g * 512 : (g + 1) * 512].T.astype(np.float16).reshape(4, 128, D)
        )
        in_maps.append(
            {
                "xs": xs,
                "wq": wq,
                "wk": wk,
                "wv": wv,
                "wo": wo,
                "invtot": invtot,
                "mask2": mask2,
                "ident": ident,
            }
        )
    return in_maps


def kernel(x, W_qkv, W_o, trace=False):
    from concourse import bass_utils

    nc = _build()
    in_maps = _host_inputs(x, W_qkv, W_o)
    res = bass_utils.run_bass_kernel_spmd(
        nc, in_maps, core_ids=list(range(8)), trace=trace
    )
    results = res.results

    f32 = np.float32
    y = np.empty((4, T, D), dtype=f32)
    fm = np.empty((4, 16, DH, DH), dtype=f32)
    for core in range(8):
        b, g = core // 2, core % 2
        if g == 0:
            y[b] = results[core]["ypart"]
        else:
            y[b] += results[core]["ypart"]
        fm[b, g * 8 : (g + 1) * 8] = results[core]["fmat"]
    fc = np.full((4, 16, 1, 1), float(T), dtype=f32)
    if trace:
        kernel._last_result = res
    return (y, fm, fc)


# revision 22
# speedup vs baseline: 1.1189x; 1.0051x over previous
"""Binary associative memory (causal linear attention with binarized k/v).

Self-contained Trainium2 Bass kernel.

Math: the reference's chunked prefix recurrence telescopes to exact causal
linear attention:
    out[t] = (1/(8*(t+1))) * sum_{s<=t} (q[t].k[s]) v[s],   k,v = sign(qkv)
    y      = out @ W_o.T   (summed over head features)
    final_matrix[b,h] = sum_t k[t] (x) v[t]   (exact integers)
so we are free to re-chunk at 128 tokens (partition width).

Sharding: 8 cores = 4 batches x 2 head-groups (8 heads each).

Precision: k/v binarize must match the fp32 reference's signs, so the k/v
projections use an fp16 double-double trick: x ~= x1 + x2 and W ~= w1 + w2
(fp16 splits, prepared on host) stacked along the contraction dim, so one
K=2048 accumulation computes (x1+x2)@(w1+w2) with ~1e-7 relative error at
full PE speed. q (continuous) uses a single fp16 term; o_proj runs fp16.

Layouts per core:
  - q,k projected feature-on-partition ([feat, tok]) for scoresT/crossT
  - v projected token-on-partition directly ([tok, vfeat]) for intra/ckv
  - k transposed per chunk on the PE (with identity) for ckv
  - state S[kf, df] fp16 (exact small integers), 2 heads per 128 partitions
"""

import functools

import numpy as np

T = 4096
D = 1024
HLOC = 8  # heads per core
DH = 64
CH = 128  # chunk size
NCH = T // CH  # 32
TT = 512  # projection token tile
NTT = T // TT  # 8

PAIRS = [(0, 2), (1, 3), (4, 6), (5, 7)]  # scores PSUM-tile pairs (same row-group)
# 3-term fp16 split of x@W: x1w1 + x2w1 + x1w2 (chunk indices into [hi(8), lo(8)])
SPLIT3 = [(wc, xc) for wc, xc in
          [(c, c) for c in range(8)] + [(c, c + 8) for c in range(8)] + [(c + 8, c) for c in range(8)]]


@functools.lru_cache(maxsize=1)
def _build():
    from contextlib import ExitStack

    import concourse.bacc as bacc
    import concourse.mybir as mybir
    import concourse.tile as tile

    f32 = mybir.dt.float32
    f16 = mybir.dt.float16

    nc = bacc.Bacc("TRN2", target_bir_lowering=False, debug=False, num_devices=8)

    # x split [x1; x2], chunk-major: [16 kc][128][T]
    xs = nc.dram_tensor("xs", [16, 128, T], f16, kind="ExternalInput").ap()
    wq = nc.dram_tensor("wq", [8, 128, 512], f16, kind="ExternalInput").ap()
    wk = nc.dram_tensor("wk", [16, 128, 512], f16, kind="ExternalInput").ap()
    wv = nc.dram_tensor("wv", [16, 128, 512], f16, kind="ExternalInput").ap()
    wo = nc.dram_tensor("wo", [4, 128, 1024], f16, kind="ExternalInput").ap()
    invtot = nc.dram_tensor("invtot", [128, T], f32, kind="ExternalInput").ap()
    mask2 = nc.dram_tensor("mask2", [128, 512], f32, kind="ExternalInput").ap()
    ident = nc.dram_tensor("ident", [128, 128], f16, kind="ExternalInput").ap()
    ypart = nc.dram_tensor("ypart", [T, D], f32, kind="ExternalOutput").ap()
    fmat = nc.dram_tensor("fmat", [HLOC, DH, DH], f32, kind="ExternalOutput").ap()

    with tile.TileContext(nc) as tc, ExitStack() as ctx:
        const = ctx.enter_context(tc.tile_pool(name="const", bufs=1))
        xpool = ctx.enter_context(tc.tile_pool(name="xp", bufs=3))
        qkpool = ctx.enter_context(tc.tile_pool(name="qk", bufs=2))
        tokpool = ctx.enter_context(tc.tile_pool(name="tok", bufs=3))
        stpool = ctx.enter_context(tc.tile_pool(name="st", bufs=8))
        opool = ctx.enter_context(tc.tile_pool(name="op", bufs=3))
        ypool = ctx.enter_context(tc.tile_pool(name="yp", bufs=3))
        spool = ctx.enter_context(tc.tile_pool(name="sp", bufs=2))
        pspool = ctx.enter_context(tc.tile_pool(name="ps", bufs=8, space="PSUM"))

        # weights needed by the first projections: emit DMAs first
        # (wk feeds the leading pk matmuls, so it goes before wq)
        wk_sb = const.tile([128, 16, 512], f16)
        for kc in range(16):
            nc.sync.dma_start(wk_sb[:, kc, :], wk[kc])
        wq_sb = const.tile([128, 8, 512], f16)
        for kc in range(8):
            nc.sync.dma_start(wq_sb[:, kc, :], wq[kc])

        xs_tiles = {}

        def dma_x(tt, kc_lo, kc_hi):
            if tt >= NTT:
                return
            if tt not in xs_tiles:
                xs_tiles[tt] = xpool.tile([128, 16, TT], f16, tag="xs", name="xs_sb")
            t0 = tt * TT
            for kc in range(kc_lo, kc_hi):
                nc.sync.dma_start(xs_tiles[tt][:, kc, :], xs[kc, :, t0 : t0 + TT])

        qk_tiles = {}

        def proj_qk(tt, fb):
            """Build projection emitters for q/k feature-block fb of tile tt.

            Returns (stream, finish): stream is a list of matmul thunks to
            interleave with other PSUM banks; finish emits the evacuations."""
            if tt >= NTT:
                return [], lambda: None
            if tt not in qk_tiles:
                qk_tiles[tt] = (
                    qkpool.tile([128, 4 * TT], f16, tag="qT", name="qT_sb"),
                    qkpool.tile([128, 4 * TT], f16, tag="kT", name="kT_sb"),
                )
            qT, kT = qk_tiles[tt]
            xt = xs_tiles[tt]
            pq = pspool.tile([128, 512], f32, tag="work", name="pq")
            pk = pspool.tile([128, 512], f32, tag="work", name="pk")

            def mm_pq(kc, pq=pq, xt=xt, fb=fb):
                nc.tensor.matmul(
                    pq,
                    wq_sb[:, kc, fb * 128 : (fb + 1) * 128],
                    xt[:, kc, :],
                    start=(kc == 0),
                    stop=(kc == 7),
                )

            def mm_pk(i, pk=pk, xt=xt, fb=fb):
                wc, xc = SPLIT3[i]
                nc.tensor.matmul(
                    pk,
                    wk_sb[:, wc, fb * 128 : (fb + 1) * 128],
                    xt[:, xc, :],
                    start=(i == 0),
                    stop=(i == len(SPLIT3) - 1),
                )

            # order pk/pq so adjacent emissions hit alternating PSUM banks
            # once merged with the pv stream: [pk*16, (pk,pq)*8]
            stream = [lambda i=i: mm_pk(i) for i in range(16)]
            for i in range(8):
                stream.append(lambda i=i: mm_pk(16 + i))
                stream.append(lambda i=i: mm_pq(i))

            def finish(qT=qT, kT=kT, pq=pq, pk=pk, fb=fb):
                nc.scalar.sign(kT[:, fb * TT : (fb + 1) * TT], pk)
                nc.scalar.copy(qT[:, fb * TT : (fb + 1) * TT], pq)

            return stream, finish

        # prologue: x(0) + full projection of tile 0 (fb pairs interleaved
        # so adjacent matmuls target different PSUM banks)
        dma_x(0, 0, 16)
        wv_sb = const.tile([128, 16, 512], f16)
        for kc in range(16):
            nc.sync.dma_start(wv_sb[:, kc, :], wv[kc])
        for fbp in range(2):
            s_a, f_a = proj_qk(0, 2 * fbp)
            s_b, f_b = proj_qk(0, 2 * fbp + 1)
            for a, b in zip(s_a, s_b):
                a()
                b()
            f_a()
            f_b()
        dma_x(1, 0, 16)

        # remaining constants (deferred so they queue behind the hot DMAs)
        wo_sb = const.tile([128, 4, 1024], f16)
        nc.sync.dma_start(wo_sb, wo.rearrange("jc p i -> p jc i"))
        inv_sb = const.tile([128, T], f32)
        nc.sync.dma_start(inv_sb, invtot)
        mask_sb = const.tile([128, 512], f32)
        nc.sync.dma_start(mask_sb, mask2)
        id_sb = const.tile([128, 128], f16)
        nc.sync.dma_start(id_sb, ident)

        # running state S[kf, df] fp16 (exact ints), 2 heads stacked per tile
        s_cur = spool.tile([128, 256], f16, tag="S", name="s_init")
        nc.vector.memset(s_cur, 0.0)

        prev_osb = None

        def oproj_bigs(m, osb_m):
            """Return N=512 matmul thunks + finisher for chunk m's o_proj."""
            ysb = ypool.tile([128, 1024], f32, tag="ysb")
            yps = [
                pspool.tile([128, 512], f32, tag="work", name=f"yp{icol}")
                for icol in range(2)
            ]
            thunks = []
            for hp in range(4):
                for icol in range(2):
                    def mm_y(hp=hp, icol=icol):
                        nc.tensor.matmul(
                            yps[icol],
                            osb_m[:, hp * 128 : (hp + 1) * 128],
                            wo_sb[:, hp, icol * 512 : (icol + 1) * 512],
                            start=(hp == 0),
                            stop=(hp == 3),
                        )
                    thunks.append(mm_y)

            def finish():
                for icol in range(2):
                    nc.scalar.copy(ysb[:, icol * 512 : (icol + 1) * 512], yps[icol])
                nc.sync.dma_start(ypart[m * CH : (m + 1) * CH, :], ysb)

            return thunks, finish

        def emit_oproj(m, osb_m):
            thunks, finish = oproj_bigs(m, osb_m)
            for t in thunks:
                t()
            finish()

        prev_osb = None
        for n in range(NCH):
            tt, ci = n // 4, n % 4
            qT, kT = qk_tiles[tt]
            xt = xs_tiles[tt]
            c0 = ci * CH
            dma_x(tt + 2, ci * 4, ci * 4 + 4)

            # --- scoresT[j, i] + k transposes. scs2[0] holds row-group-0
            # heads (0,2,4,6), scs2[1] the row-group-64 heads: same-bank
            # matmuls must share a PE row-group (concurrent different-row
            # matmuls on one bank collide fatally); the two banks overlap. ---
            scs2 = [
                pspool.tile([128, 512], f32, tag="work", name=f"scs{i}")
                for i in range(2)
            ]
            ktp = pspool.tile([128, 512], f16, tag="work", name="ktp")

            def mm_score(h):
                bp = (h % 2) * 64
                hp = h // 2
                soff = (h // 4) * 256 + ((h // 2) % 2) * 128
                sl = slice(hp * TT + c0, hp * TT + c0 + CH)
                nc.tensor.matmul(
                    scs2[h % 2][:, soff : soff + 128],
                    kT[bp : bp + 64, sl],
                    qT[bp : bp + 64, sl],
                    start=True,
                    stop=True,
                )

            for h in range(4):
                mm_score(h)
            for hp in range(4):
                mm_score(4 + hp)
                sl = slice(hp * TT + c0, hp * TT + c0 + CH)
                nc.tensor.transpose(
                    ktp[:, hp * 128 : (hp + 1) * 128], kT[:, sl], id_sb
                )
            sts = []
            for i in range(2):
                st = stpool.tile([128, 512], f16, tag="st")
                nc.vector.tensor_mul(st, scs2[i], mask_sb)
                sts.append(st)
            ktok = tokpool.tile([128, 512], f16, tag="ktok")
            nc.vector.tensor_copy(ktok, ktp)

            # --- merged projection stream: v (this chunk) + q/k of the
            # next tile, alternating PSUM banks to hide drain bubbles ---
            pv = pspool.tile([128, 512], f32, tag="work", name="pv")

            def mm_pv(i):
                wc, xc = SPLIT3[i]
                nc.tensor.matmul(
                    pv,
                    xt[:, xc, c0 : c0 + CH],
                    wv_sb[:, wc, :],
                    start=(i == 0),
                    stop=(i == len(SPLIT3) - 1),
                )

            qk_stream, qk_finish = proj_qk(tt + 1, ci)
            # part A: all of pv + the first 24 qk matmuls, banks alternating;
            # the last 8 qk matmuls are held back to re-densify the PE after
            # the ckv/ot smalls (keeps every HAM window busy)
            for i in range(24):
                mm_pv(i)
                if i < min(24, len(qk_stream)):
                    qk_stream[i]()
            vtok = tokpool.tile([128, 512], f16, tag="vtok")
            nc.scalar.sign(vtok, pv)

            # --- ckv[kf, df] + outT = intraT + crossT, interleaved ---
            ckv = pspool.tile([128, 256], f32, tag="work", name="ckv")
            ot = pspool.tile([128, 512], f32, tag="work", name="ot")
            for h in range(8):
                bp = (h % 2) * 64
                hp = h // 2
                soff = (h // 4) * 256 + ((h // 2) % 2) * 128
                nc.tensor.matmul(
                    ot[bp : bp + 64, hp * 128 : hp * 128 + 128],
                    vtok[:, h * 64 : h * 64 + 64],
                    sts[h % 2][:, soff : soff + 128],
                    start=True,
                    stop=(n == 0),
                    tile_position=(0, bp),
                )
                if n > 0:
                    nc.tensor.matmul(
                        ot[bp : bp + 64, hp * 128 : hp * 128 + 128],
                        s_cur[bp : bp + 64, hp * 64 : hp * 64 + 64],
                        qT[bp : bp + 64, hp * TT + c0 : hp * TT + c0 + CH],
                        start=False,
                        stop=True,
                        tile_position=(bp, bp),
                    )
                nc.tensor.matmul(
                    ckv[bp : bp + 64, (h // 2) * 64 : (h // 2) * 64 + 64],
                    ktok[:, h * 64 : h * 64 + 64],
                    vtok[:, h * 64 : h * 64 + 64],
                    start=True,
                    stop=True,
                    tile_position=(0, bp),
                )

            # --- part B of the projection + o_proj of the previous chunk:
            # dense N=512 work splitting the ckv/ot smalls from the next
            # chunk's scores smalls, so no HAM window goes idle ---
            for i in range(24, len(qk_stream)):
                qk_stream[i]()
            qk_finish()
            if prev_osb is not None:
                emit_oproj(n - 1, prev_osb)
                prev_osb = None

            # --- state update + out scale (DVE) ---
            s_new = spool.tile([128, 256], f16, tag="S", name="s_new")
            nc.vector.tensor_add(s_new, s_cur, ckv)
            s_cur = s_new
            osb = opool.tile([128, 512], f16, tag="osb")
            for hp in range(4):
                nc.vector.tensor_mul(
                    osb[:, hp * 128 : (hp + 1) * 128],
                    ot[:, hp * 128 : (hp + 1) * 128],
                    inv_sb[:, n * CH : n * CH + CH],
                )
            prev_osb = osb

        emit_oproj(NCH - 1, prev_osb)

        # --- final matrix (exact integer sums) ---
        fsb = opool.tile([128, 256], f32, tag="fsb")
        nc.vector.tensor_copy(fsb, s_cur)
        for h in range(8):
            nc.sync.dma_start(
                fmat[h],
                fsb[(h % 2) * 64 : (h % 2) * 64 + 64, (h // 2) * 64 : (h // 2) * 64 + 64],
            )

    nc.compile()
    return nc


def _split16(a):
    """fp16 double-double split along axis 0-stacking: returns (hi, lo)."""
    hi = a.astype(np.float16)
    lo = (a - hi.astype(np.float32)).astype(np.float16)
    return hi, lo


def _host_inputs(x, W_qkv, W_o):
    f32 = np.float32
    tvec = np.arange(1, T + 1, dtype=np.float64)
    inv = (1.0 / (8.0 * tvec)).astype(f32)
    invtot = np.ascontiguousarray(np.broadcast_to(inv[None, :], (128, T)))
    jj = np.arange(128)
    maskT = (jj[:, None] <= jj[None, :]).astype(f32)
    mask2 = np.ascontiguousarray(np.tile(maskT, (1, 4)))
    ident = np.eye(128, dtype=np.float16)

    Wq3 = np.asarray(W_qkv, dtype=f32).reshape(3, 16, DH, D)
    in_maps = []
    for core in range(8):
        b, g = core // 2, core % 2
        xT = np.asarray(x[b], dtype=f32).T  # [D, T]
        x1, x2 = _split16(xT)
        xs = np.ascontiguousarray(
            np.concatenate([x1, x2], axis=0).reshape(16, 128, T)
        )
        # weight column blocks for this head group, [D, 512] each
        wqc = Wq3[0, g * 8 : (g + 1) * 8].reshape(512, D).T
        wkc = Wq3[1, g * 8 : (g + 1) * 8].reshape(512, D).T
        wvc = Wq3[2, g * 8 : (g + 1) * 8].reshape(512, D).T
        wq = np.ascontiguousarray(wqc.astype(np.float16).reshape(8, 128, 512))
        wk1, wk2 = _split16(wkc)
        wk = np.ascontiguousarray(np.concatenate([wk1, wk2], axis=0).reshape(16, 128, 512))
        wv1, wv2 = _split16(wvc)
        wv = np.ascontiguousarray(np.concatenate([wv1, wv2], axis=0).reshape(16, 128, 512))
        wo = np.ascontiguousarray(
            np.asarray(W_o, dtype=f32)[:, g * 512 : (g + 1) * 512].T.astype(np.float16).reshape(4, 128, D)
        )
        in_maps.append(
            {
                "xs": xs,
                "wq": wq,
                "wk": wk,
                "wv": wv,
                "wo": wo,
                "invtot": invtot,
                "mask2": mask2,
                "ident": ident,
            }
        )
    return in_maps


def kernel(x, W_qkv, W_o, trace=False):
    from concourse import bass_utils

    nc = _build()
    in_maps = _host_inputs(x, W_qkv, W_o)
    res = bass_utils.run_bass_kernel_spmd(
        nc, in_maps, core_ids=list(range(8)), trace=trace
    )
    results = res.results

    f32 = np.float32
    y = np.empty((4, T, D), dtype=f32)
    fm = np.empty((4, 16, DH, DH), dtype=f32)
    for core in range(8):
        b, g = core // 2, core % 2
        if g == 0:
            y[b] = results[core]["ypart"]
        else:
            y[b] += results[core]["ypart"]
        fm[b, g * 8 : (g + 1) * 8] = results[core]["fmat"]
    fc = np.full((4, 16, 1, 1), float(T), dtype=f32)
    if trace:
        kernel._last_result = res
    return (y, fm, fc)
